# revision 15
# baseline (speedup 1.0000x reference)
"""Trainium2 Bass kernel for nn_Clustering (vq_codebook).

Math facts used (derived from the reference):
  - V is never used by the reference.
  - mean(softmax(.)) == 1/C exactly, so mu == x == 0.2 everywhere:
      * the (x-mu) term of log_prob vanishes (diff ~1e-14, far below fp32 eps)
      * cluster_k / W_k / b_k are never needed
      * the CE term is the constant 51.2*ln(256)
    loss = mean(ln sigma) + 0.5*ln(2*pi) + 51.2*ln(256),
    sigma = softplus(unbiased-std over C of softmax_q).
  - The faithful reshape of the unfold means row (l'=32h+q, u') of the
    [256, 50, 512] per-b activation matrix is a flat reinterpretation of the
    (s, d, u) block K_padded[b+u, h, 8q+s, d] (f = 3200 s + 50 d + u).
  - centers[i, l', c] = (1/50) sum_u softmax_q[l', u, c] * (argmax_c != i).

Device (8 cores, data-parallel over b; CE would couple l' across cores but it
is constant, so b-sharding keeps every reduction core-local):
  per core: X^T[u', m, l'] @ W_q -> gelu -> exp -> row stats -> argmax via
  masked-iota min -> masked sums over u' -> centers [5, 256, 5] + ln(sigma)
  partial sums.
Host: tiny projection centers @ W_back (52 MFLOP), exact-erf gelu, faithful
reshape, max over the 5 codebook slots; plus the loss constant.
"""

import math

import numpy as np

import concourse.bass as bass
import concourse.mybir as mybir
from concourse import bass_utils
from concourse.tile import TileContext

B, H, L, DK = 8, 8, 256, 64
C = 5
DM = H * DK  # 512
U = 10 * C  # 50
P = 128
F32 = mybir.dt.float32
AX = mybir.AxisListType.X
ALU = mybir.AluOpType
ACT = mybir.ActivationFunctionType

_CACHE = {}


def _fix_waits(bj: bytes) -> bytes:
    """This compiler build fits only ONE sync-wait per instruction
    ("Too many sync wait commands"). Split every multi-wait instruction by
    hoisting the extra waits onto same-engine NoOps inserted right before it
    (no reordering, so dependency semantics are unchanged)."""
    import json as _json
    d = _json.loads(bj)
    n = 0
    for fn in d["functions"]:
        blocks = fn.get("blocks") or [fn]
        for blk in blocks:
            ins = blk.get("instructions")
            if not ins:
                continue
            out = []
            for inst in ins:
                si = inst.get("sync_info") or {}
                w = si.get("on_wait") or []
                if len(w) > 1:
                    for extra in w[:-1]:
                        n += 1
                        out.append({
                            "name": f"I-wfix-{n}",
                            "opcode": "NoOp",
                            "engine": inst["engine"],
                            "ins": [],
                            "outs": [],
                            "debug": inst.get("debug", 0),
                            "sync_info": {"on_wait": [extra], "on_update": []},
                        })
                    si["on_wait"] = [w[-1]]
                out.append(inst)
            blk["instructions"] = out
    return _json.dumps(d).encode()


def _build_nc():
    nc = bass.Bass()
    XT = nc.dram_tensor("XT", [U, DM, L], F32, kind="ExternalInput")  # [u', m, l']
    WQ = nc.dram_tensor("WQ", [DM, C], F32, kind="ExternalInput")
    BIASQ = nc.dram_tensor("BIASQ", [P, 10 * C], F32, kind="ExternalInput")
    IOTA = nc.dram_tensor("IOTA", [P, U * C], F32, kind="ExternalInput")
    CEN = nc.dram_tensor("CEN", [C, 2, P, C], F32, kind="ExternalOutput")
    LS = nc.dram_tensor("LS", [P, 1], F32, kind="ExternalOutput")

    with TileContext(nc) as tc:
        with (
            tc.tile_pool(name="const", bufs=1) as cpool,
            tc.tile_pool(name="xt", bufs=8) as xpool,
            tc.tile_pool(name="big", bufs=2) as bpool,
            tc.tile_pool(name="small", bufs=3) as spool,
            tc.tile_pool(name="psum", bufs=3, space="PSUM") as ppool,
        ):
            wq_t = cpool.tile([P, 4, C], F32)
            nc.gpsimd.dma_start(wq_t, WQ.rearrange("(a p) c -> p a c", p=P))
            # PE's weight-load slot fits only ONE sync wait in this compiler
            # build, so every PE input must arrive via the vector engine: all
            # PE waits then coalesce into a single DVE-semaphore wait.
            wqc = cpool.tile([P, 4, C], F32)
            nc.vector.tensor_copy(wqc, wq_t)
            bias_t = cpool.tile([P, 10 * C], F32)
            nc.gpsimd.dma_start(bias_t, BIASQ[:])
            iota_t = cpool.tile([P, U * C], F32)
            nc.gpsimd.dma_start(iota_t, IOTA[:])

            G0 = bpool.tile([P, U * C], F32, tag="G0")
            G1 = bpool.tile([P, U * C], F32, tag="G1")
            G = [G0, G1]

            # ---- matmul: A[l', (u,c)] = X @ W_q + b_q, then gelu ----
            for ug in range(5):  # groups of 10 u'
                ps0 = ppool.tile([P, 10 * C], F32, tag="ps0")
                ps1 = ppool.tile([P, 10 * C], F32, tag="ps1")
                ps = [ps0, ps1]
                for uu in range(10):
                    u = ug * 10 + uu
                    xts = []
                    for mb in range(4):
                        xt = xpool.tile([P, L], F32, tag=f"xt{mb}")
                        nc.gpsimd.dma_start(xt, XT[u, mb * P:(mb + 1) * P, :])
                        xtc = xpool.tile([P, L], F32, tag=f"xtc{mb}")
                        nc.vector.tensor_copy(xtc, xt)
                        xts.append(xtc)
                    for half in range(2):
                        for mb in range(4):
                            nc.tensor.matmul(
                                ps[half][:, uu * C:(uu + 1) * C],
                                xts[mb][:, half * P:(half + 1) * P],
                                wqc[:, mb, :],
                                start=(mb == 0),
                                stop=(mb == 3),
                            )
                for half in range(2):
                    tmp = spool.tile([P, 10 * C], F32, tag="btmp")
                    nc.vector.tensor_add(tmp, ps[half], bias_t)
                    nc.scalar.activation(
                        G[half][:, ug * 10 * C:(ug + 1) * 10 * C], tmp, ACT.Gelu
                    )

            # ---- per-half stats / argmax / centers ----
            ls_acc = cpool.tile([P, 2], F32)
            for half in range(2):
                E = bpool.tile([P, U * C], F32, tag="E")
                nc.scalar.activation(E, G[half], ACT.Exp)
                Ev = E[:].rearrange("p (u c) -> p u c", c=C)

                S = spool.tile([P, U], F32, tag="S")
                nc.vector.reduce_sum(S, Ev, axis=AX)
                R = spool.tile([P, U], F32, tag="R")
                nc.vector.reciprocal(R, S)

                E2 = bpool.tile([P, U * C], F32, tag="E2")
                nc.vector.tensor_mul(E2, E, E)
                S2 = spool.tile([P, U], F32, tag="S2")
                nc.vector.reduce_sum(S2, E2[:].rearrange("p (u c) -> p u c", c=C), axis=AX)
                # Q2 = S2 * R * R ; var = 0.25*Q2 - 0.05 ; guard >= 0 ; std; softplus; ln
                q2 = spool.tile([P, U], F32, tag="q2")
                nc.vector.tensor_mul(q2, S2, R)
                nc.vector.tensor_mul(q2, q2, R)
                var = spool.tile([P, U], F32, tag="var")
                nc.vector.tensor_scalar(var, q2, 0.25, -0.05, ALU.mult, ALU.add)
                nc.vector.tensor_scalar_max(var, var, 0.0)
                std = spool.tile([P, U], F32, tag="std")
                nc.scalar.activation(std, var, ACT.Sqrt)
                # ln(sigma) = ln(softplus(std)) = ln(ln(1 + exp(std)))
                es = spool.tile([P, U], F32, tag="es")
                nc.scalar.activation(es, std, ACT.Exp)
                nc.vector.tensor_scalar_add(es, es, 1.0)
                sig = spool.tile([P, U], F32, tag="sig")
                nc.scalar.activation(sig, es, ACT.Ln)
                lsg = spool.tile([P, U], F32, tag="lsg")
                nc.scalar.activation(lsg, sig, ACT.Ln)
                nc.vector.reduce_sum(ls_acc[:, half:half + 1], lsg, axis=AX)

                # argmax (first max index) via masked-iota min
                m = spool.tile([P, U], F32, tag="m")
                nc.vector.reduce_max(m, Ev, axis=AX)
                m5 = bpool.tile([P, U * C], F32, tag="m5")
                m5v = m5[:].rearrange("p (u c) -> p u c", c=C)
                for c in range(C):
                    nc.vector.tensor_copy(m5v[:, :, c], m)
                cmp = bpool.tile([P, U * C], F32, tag="cmp")
                nc.vector.tensor_tensor(cmp, E, m5, ALU.is_equal)
                val = bpool.tile([P, U * C], F32, tag="val")
                nc.vector.scalar_tensor_tensor(
                    val, cmp, -1e6, iota_t, ALU.mult, ALU.add
                )
                ind = spool.tile([P, U], F32, tag="ind")
                nc.vector.tensor_reduce(
                    ind, val[:].rearrange("p (u c) -> p u c", c=C),
                    op=ALU.min, axis=AX,
                )

                # centers_i = sum_u E * R * (ind != i)
                for i in range(C):
                    t = spool.tile([P, U], F32, tag="t")
                    nc.vector.tensor_scalar_add(t, ind, float(-i))
                    nc.vector.tensor_mul(t, t, t)
                    nc.vector.tensor_scalar_min(t, t, 1.0)
                    nc.vector.tensor_mul(t, t, R)
                    w5 = bpool.tile([P, U * C], F32, tag="w5")
                    w5v = w5[:].rearrange("p (u c) -> p u c", c=C)
                    for c in range(C):
                        nc.vector.tensor_copy(w5v[:, :, c], t)
                    msk = bpool.tile([P, U * C], F32, tag="msk")
                    nc.vector.tensor_mul(msk, E, w5)
                    cen = spool.tile([P, C], F32, tag="cen")
                    nc.vector.reduce_sum(
                        cen, msk[:].rearrange("p (u c) -> p c u", c=C), axis=AX
                    )
                    nc.gpsimd.dma_start(CEN[i, half], cen)

            ls1 = cpool.tile([P, 1], F32)
            nc.vector.tensor_add(ls1, ls_acc[:, 0:1], ls_acc[:, 1:2])
            nc.gpsimd.dma_start(LS[:], ls1)
    return nc


def _gelu_np(x):
    from scipy.special import erf
    return 0.5 * x * (1.0 + erf(x / np.sqrt(2.0).astype(np.float32)))


def kernel(K, V, W_k, b_k, W_q, b_q, W_back, b_back):
    K = np.asarray(K, dtype=np.float32)
    W_q = np.asarray(W_q, dtype=np.float32)
    b_q = np.asarray(b_q, dtype=np.float32)
    W_back = np.asarray(W_back, dtype=np.float32)
    b_back = np.asarray(b_back, dtype=np.float32)

    if "nc" not in _CACHE:
        nc_new = _build_nc()
        fixed = _fix_waits(nc_new.to_json_bytes())
        nc_new.to_json_bytes = lambda: fixed  # instance-level shadow
        _CACHE["nc"] = nc_new
    nc = _CACHE["nc"]

    # host-side unfold (pure data movement): X_b[l'=32h+q, u', m]
    Kp = np.concatenate([np.zeros((U, H, L, DK), np.float32), K[1:]], axis=0)
    iota = np.tile((np.arange(C, dtype=np.float32) + 1e6), (P, U))
    in_maps = []
    for b in range(B):
        win = Kp[b:b + U]                                   # [50, 8, 256, 64]
        t = win.reshape(U, H, 32, 8, DK).transpose(1, 2, 3, 4, 0)  # h,q,s,d,u
        Xb = t.reshape(L, U, DM)                            # [l'=h*32+q, u', m]
        XTb = np.ascontiguousarray(Xb.transpose(1, 2, 0))   # [u', m, l']
        in_maps.append({
            "XT": XTb,
            "WQ": W_q,
            "BIASQ": np.tile(b_q, (P, 10)).astype(np.float32),
            "IOTA": iota.astype(np.float32),
        })

    res = bass_utils.run_bass_kernel_spmd(nc, in_maps, core_ids=list(range(B)))
    _CACHE["last_results"] = res

    # gather: centers[i, b, l', c], l' = half*128 + p
    centers = np.empty((C, B, L, C), np.float32)
    ls_sum = 0.0
    for b in range(B):
        cen = res.results[b]["CEN"]                         # [5, 2, 128, 5]
        # CEN[i, half, p, c] -> l' = half*128+p  (C-order reshape of [2,128] -> 256)
        centers[:, b] = cen.reshape(C, 2 * P, C)
        ls_sum += float(res.results[b]["LS"].sum())
    centers *= np.float32(1.0 / U)

    loss = (ls_sum / (B * L * U)
            + 0.5 * math.log(2.0 * math.pi)
            + 0.2 * L * math.log(float(L)) / 1.0)

    # stage D (host, 52 MFLOP): projection + exact gelu + faithful reshape + max
    cc_big = _gelu_np(centers @ W_back + b_back)            # [5, 8, 256, 512]
    cc = cc_big.reshape(B, H, C, L, DK).max(axis=2)         # faithful reshape
    return cc.astype(np.float32), np.float32(loss)
